# revision 24
# baseline (speedup 1.0000x reference)
"""Trainium2 Bass kernel: dark-channel + 15x15 erosion (min-pool, stride 1,
+inf padding), data-parallel over 8 NeuronCores.

Input  I: [32, 3, 512, 512] f32, k: scalar (15)
Output:   [32, 1, 512, 512] f32  (min over channels, then kxk spatial min)

The host converts the input to f16 (values are uniform [0,1); min is pure
selection so f16 keeps rel err ~2e-4, far inside the 2e-2 gate) and the
kernel writes an f16 output that the host upcasts.  This halves HBM traffic
versus f32 in both directions.

Elementwise min exists ONLY on DVE (walrus rejects tensor_tensor min/max on
Pool and min as a DMA compute op), so the kernel is DVE-bound at 10 full-
image f16 passes per image (2 channel-min + 4+4 dyadic erosion stages).
Everything else is arranged to keep DVE 100% fed:

  1. The 3 channels of each image load CONCURRENTLY on the SP/ACT/Pool
     DMA queues (DMA transfer time is charged to the issuing engine, so
     the three streams run in parallel with each other and with DVE).
  2. Channel-min on DVE (per-half for image 0 to cut the kernel head).
  3. Horizontal 15-min-filter: dyadic shift-1/2/4/7 stages on DVE.
  4. PE transpose (identity matmul) into PSUM, 8 blocks per bank, one big
     ScalarE evac per bank -> column layout.
  5. Vertical filter on DVE; for the last two images the final stage is
     split per 128-row slice so the transpose-back/evac/store tail
     pipelines against it.
  6. PE transpose back, ScalarE evacs (the last image's final two run on
     the by-then-idle DVE), f16 stores on SP.

The walrus backend encodes at most ONE sync-wait per instruction and fails
codegen with "Too many sync wait commands" otherwise, while Tile freely
emits several (pool slot reuse, kernel-tail drain).  The post-pass at the
end of _build_nc hoists all but one wait of every instruction onto
single-wait NOPs inserted right before it on the same engine - identical
semantics (the engine sequencer performs the waits in order), and every
instruction then fits the encoding.  CoreSim cannot execute the inserted
NOPs, so the simulator path builds with split_waits=False.
"""

import sys

if "/opt/trn_rl_repo" not in sys.path:
    sys.path.insert(0, "/opt/trn_rl_repo")

import numpy as np

N_CORES = 8
IMGS = 4          # images per core
C = 3
H = W = 512
K = 15
PAD = K // 2      # 7
L = 8             # left pad in filter buffers (>= PAD+1)
PITCH = L + 512 + 8   # 528, padded row/col length
NJ = H // 128     # row tiles
NB = W // 128     # col blocks
PADV = 30000.0    # effective +inf for data in [0,1)

_cache = {}


def _build_nc(split_waits=True):
    import concourse.bass as bass
    import concourse.mybir as mybir
    import concourse.tile as tile
    import concourse.masks as masks

    F16 = mybir.dt.float16
    MIN = mybir.AluOpType.min

    nc = bass.Bass("TRN2", target_bir_lowering=False, debug=False)
    inp = nc.dram_tensor("inp", [IMGS, C, H, W], F16, kind="ExternalInput")
    out = nc.dram_tensor("out", [IMGS, 1, H, W], F16, kind="ExternalOutput")

    with tile.TileContext(nc) as tc:
        with (
            tc.tile_pool(name="const", bufs=1) as cpool,
            tc.tile_pool(name="xpool", bufs=3) as xpool,
            tc.tile_pool(name="ypool", bufs=2) as ypool,
            tc.tile_pool(name="work", bufs=2) as work,
            tc.tile_pool(name="resp", bufs=2) as resp,
            tc.tile_pool(name="vbp", bufs=2) as vbp,
            tc.tile_pool(name="vwork", bufs=2) as vwork,
            tc.tile_pool(name="vresp", bufs=2) as vresp,
            tc.tile_pool(name="opool", bufs=2) as opool,
            tc.tile_pool(name="psum", bufs=3, space="PSUM") as psum,
            tc.tile_pool(name="psum2", bufs=2, space="PSUM") as psum2,
        ):
            ident = cpool.tile([128, 128], F16)
            masks.make_identity(nc, ident[:])
            # pre-warm the ACT activation table (Copy) off the critical path
            warm = cpool.tile([128, 2], F16)
            nc.scalar.copy(warm[:, 0:1], ident[:, 0:1])

            def src_ap(i, c, hh):
                return inp[i, c, 256 * hh : 256 * (hh + 1)].rearrange(
                    "(j p) w -> p j w", p=128
                )

            def loads(i):
                """the 3 channels land concurrently on SP/ACT/Pool queues."""
                xp = xpool.tile([128, NJ, PITCH], F16, tag="xp", name="xp")
                nc.gpsimd.memset(xp[:, :, 0:L], PADV)
                nc.gpsimd.memset(xp[:, :, L + W : PITCH], PADV)
                y = ypool.tile([128, NJ, W], F16, tag="y", name="y")
                z = ypool.tile([128, NJ, W], F16, tag="z", name="z")
                for hh in range(2):
                    jj = slice(2 * hh, 2 * (hh + 1))
                    nc.sync.dma_start(xp[:, jj, L : L + W], src_ap(i, 0, hh))
                    nc.scalar.dma_start(y[:, jj, :], src_ap(i, 1, hh))
                    nc.gpsimd.dma_start(z[:, jj, :], src_ap(i, 2, hh))
                return xp, y, z

            def min_chain(i, src, wpool, rpool, rtag, yz=None,
                          split_final=False):
                """15-wide min filter along last dim of src [128, n, PITCH];
                logical x at [L : L+512].  Returns [128, n, 512] f16."""
                n = src.shape[1]
                if yz is not None:
                    # channel-min; per-half for image 0 so DVE starts as
                    # soon as the first half of the loads lands
                    y, z = yz
                    halves = range(2) if i == 0 else [slice(None)]
                    for hh in halves:
                        jj = (slice(2 * hh, 2 * (hh + 1))
                              if isinstance(hh, int) else hh)
                        nc.vector.tensor_tensor(
                            src[:, jj, L : L + W], src[:, jj, L : L + W],
                            y[:, jj, :], op=MIN,
                        )
                        nc.vector.tensor_tensor(
                            src[:, jj, L : L + W], src[:, jj, L : L + W],
                            z[:, jj, :], op=MIN,
                        )
                s1 = wpool.tile([128, n, PITCH], F16, tag=rtag + "a", name="s1")
                nc.vector.tensor_tensor(
                    s1[:, :, 0:526], src[:, :, 0:526], src[:, :, 1:527], op=MIN
                )
                s2 = wpool.tile([128, n, PITCH], F16, tag=rtag + "b", name="s2")
                nc.vector.tensor_tensor(
                    s2[:, :, 0:524], s1[:, :, 0:524], s1[:, :, 2:526], op=MIN
                )
                s4 = wpool.tile([128, n, PITCH], F16, tag=rtag + "c", name="s4")
                nc.vector.tensor_tensor(
                    s4[:, :, 0:520], s2[:, :, 0:520], s2[:, :, 4:524], op=MIN
                )
                res = rpool.tile([128, n, W], F16, tag=rtag, name="res")
                if split_final:
                    # final combine per 128-slice so downstream per-slice
                    # consumers (t2_out) start before the whole image is done
                    for j in range(NJ):
                        nc.vector.tensor_tensor(
                            res[:, :, 128 * j : 128 * (j + 1)],
                            s4[:, :, 128 * j + 1 : 128 * j + 129],
                            s4[:, :, 128 * j + 8 : 128 * j + 136],
                            op=MIN,
                        )
                else:
                    nc.vector.tensor_tensor(
                        res[:], s4[:, :, 1:513], s4[:, :, 8:520], op=MIN
                    )
                return res

            def make_vb2():
                """column-layout tile holding a PAIR of images, so the
                v-filter processes two images per DVE op."""
                vb2 = vbp.tile([128, 2, NB, PITCH], F16, tag="vb2",
                               name="vb2")
                nc.gpsimd.memset(vb2[:, :, :, 0:L], PADV)
                nc.gpsimd.memset(vb2[:, :, :, L + H : PITCH], PADV)
                return vb2

            def t1(i, res, vb2, hh):
                """row layout -> column layout into half hh of vb2.
                Two 8-block PSUM banks -> two big evacs (less ACT init)."""
                for bp in range(2):
                    pt = psum.tile([128, 2, NJ, 128], F16, tag="pt", name="pt")
                    for bb in range(2):
                        b = 2 * bp + bb
                        for j in range(NJ):
                            nc.tensor.transpose(
                                pt[:, bb, j, :],
                                res[:, j, 128 * b : 128 * (b + 1)],
                                ident[:],
                            )
                    nc.scalar.copy(
                        vb2[:, hh, 2 * bp : 2 * (bp + 1), L : L + H],
                        pt[:].rearrange("p a j w -> p a (j w)"),
                    )

            def t2_out(i, vres):
                """column layout -> row layout, store to HBM."""
                o = opool.tile([128, NJ, W], F16, tag="o", name="o")
                if i < IMGS - 1:
                    # two half-image banks, big evacs, half-image stores
                    for half in range(2):
                        pt = psum.tile([128, 2, NB, 128], F16, tag="pt2",
                                       name="pt2")
                        for jj in range(2):
                            j = 2 * half + jj
                            for b in range(NB):
                                nc.tensor.transpose(
                                    pt[:, jj, b, :],
                                    vres[:, b, 128 * j : 128 * (j + 1)],
                                    ident[:],
                                )
                        nc.scalar.copy(
                            o[:, 2 * half : 2 * (half + 1), :],
                            pt[:].rearrange("p a b w -> p a (b w)"),
                        )
                        nc.sync.dma_start(
                            out[i, 0, 256 * half : 256 * (half + 1)].rearrange(
                                "(j p) w -> p j w", p=128
                            ),
                            o[:, 2 * half : 2 * (half + 1), :],
                        )
                    return
                # last image: per-row-tile chains so the tail pipelines;
                # the last two evacs run on the (by now idle) DVE
                for j in range(NJ):
                    pt = psum2.tile([128, NB, 128], F16, tag="pt3", name="pt3")
                    for b in range(NB):
                        nc.tensor.transpose(
                            pt[:, b, :],
                            vres[:, b, 128 * j : 128 * (j + 1)],
                            ident[:],
                        )
                    flat = pt[:].rearrange("p n w -> p (n w)")
                    if j < 2:
                        nc.scalar.copy(o[:, j, :], flat)
                    else:
                        nc.vector.tensor_copy(o[:, j, :], flat)
                    nc.sync.dma_start(
                        out[i, 0, 128 * j : 128 * (j + 1)].rearrange(
                            "(q p) w -> p q w", p=128
                        ),
                        o[:, j : j + 1, :],
                    )

            def h_stage(i, ld, vb2, hh):
                xp, y, z = ld
                res = min_chain(i, xp, work, resp, "h", yz=(y, z))
                t1(i, res, vb2, hh)

            def v_pair(p, vb2):
                """v-filter over an image pair in one set of DVE ops."""
                s1 = vwork.tile([128, 2, NB, PITCH], F16, tag="va", name="vs1")
                nc.vector.tensor_tensor(
                    s1[:, :, :, 0:526], vb2[:, :, :, 0:526],
                    vb2[:, :, :, 1:527], op=MIN,
                )
                s2 = vwork.tile([128, 2, NB, PITCH], F16, tag="vb", name="vs2")
                nc.vector.tensor_tensor(
                    s2[:, :, :, 0:524], s1[:, :, :, 0:524],
                    s1[:, :, :, 2:526], op=MIN,
                )
                s4 = vwork.tile([128, 2, NB, PITCH], F16, tag="vc", name="vs4")
                nc.vector.tensor_tensor(
                    s4[:, :, :, 0:520], s2[:, :, :, 0:520],
                    s2[:, :, :, 4:524], op=MIN,
                )
                vres2 = vresp.tile([128, 2, NB, W], F16, tag="vres",
                                   name="vres2")
                if p == 0:
                    nc.vector.tensor_tensor(
                        vres2[:], s4[:, :, :, 1:513], s4[:, :, :, 8:520],
                        op=MIN,
                    )
                else:
                    # per-slice finals so t2's tail pipelines against them
                    for j in range(NJ):
                        nc.vector.tensor_tensor(
                            vres2[:, :, :, 128 * j : 128 * (j + 1)],
                            s4[:, :, :, 128 * j + 1 : 128 * j + 129],
                            s4[:, :, :, 128 * j + 8 : 128 * j + 136],
                            op=MIN,
                        )
                t2_out(2 * p, vres2[:, 0])
                t2_out(2 * p + 1, vres2[:, 1])

            # -- software pipeline, full skew: all H stages then all V --
            ld = [None] * IMGS
            ld[0] = loads(0)
            ld[1] = loads(1)
            vb2a = make_vb2()
            h_stage(0, ld[0], vb2a, 0)
            ld[2] = loads(2)
            h_stage(1, ld[1], vb2a, 1)
            ld[3] = loads(3)
            vb2b = make_vb2()
            h_stage(2, ld[2], vb2b, 0)
            h_stage(3, ld[3], vb2b, 1)
            v_pair(0, vb2a)
            v_pair(1, vb2b)

    if not split_waits:
        return nc
    # Post-pass: walrus encodes at most ONE sync-wait per instruction.
    # Hoist all but one wait of any multi-wait instruction onto
    # single-wait NOPs inserted just before it on the same engine
    # (identical semantics: the sequencer performs the waits in order).
    import concourse.mybir as mybir

    nsplit = 0
    for bb in nc.main_func.blocks:
        idx = 0
        while idx < len(bb.instructions):
            ins = bb.instructions[idx]
            si = ins.sync_info
            if si is not None and si.on_wait and len(si.on_wait) > 1:
                waits = list(si.on_wait)
                for w in waits[:-1]:
                    nop = mybir.InstNoOp(
                        name=f"W-split-{nsplit}", ins=[], outs=[]
                    )
                    nop.engine = ins.engine
                    nop.sync_info = mybir.SyncInfo(
                        on_wait=[w], on_update=[]
                    )
                    bb.instructions.insert(idx, nop)
                    nsplit += 1
                    idx += 1
                ins.sync_info = mybir.SyncInfo(
                    on_wait=[waits[-1]], on_update=list(si.on_update or [])
                )
            idx += 1
    return nc


def _get_nc():
    if "nc" not in _cache:
        _cache["nc"] = _build_nc()
    return _cache["nc"]


def kernel(I, k):
    from concourse.bass_utils import run_bass_kernel_spmd

    k = int(np.asarray(k))
    assert k == K, f"kernel compiled for k={K}, got {k}"
    I = np.asarray(I)
    B = I.shape[0]
    assert I.shape == (B, C, H, W) and B == N_CORES * IMGS
    I16 = np.ascontiguousarray(I.astype(np.float16))

    nc = _get_nc()
    in_maps = [
        {"inp": I16[c * IMGS : (c + 1) * IMGS]} for c in range(N_CORES)
    ]
    res = run_bass_kernel_spmd(nc, in_maps, list(range(N_CORES))).results
    return np.concatenate(
        [res[c]["out"] for c in range(N_CORES)], axis=0
    ).astype(np.float32)


# revision 26
# speedup vs baseline: 1.0315x; 1.0315x over previous
"""Trainium2 Bass kernel: dark-channel + 15x15 erosion (min-pool, stride 1,
+inf padding), data-parallel over 8 NeuronCores.

Input  I: [32, 3, 512, 512] f32, k: scalar (15)
Output:   [32, 1, 512, 512] f32  (min over channels, then kxk spatial min)

The host converts the input to f16 (values are uniform [0,1); min is pure
selection so f16 keeps rel err ~2e-4, far inside the 2e-2 gate) and the
kernel writes an f16 output that the host upcasts.  This halves HBM traffic
versus f32 in both directions.

Elementwise min exists ONLY on DVE (walrus rejects tensor_tensor min/max on
Pool and min as a DMA compute op), so the kernel is DVE-bound at 10 full-
image f16 passes per image (2 channel-min + 4+4 dyadic erosion stages).
Everything else is arranged to keep DVE 100% fed:

  1. The 3 channels of each image load CONCURRENTLY on the SP/ACT/Pool
     DMA queues (DMA transfer time is charged to the issuing engine, so
     the three streams run in parallel with each other and with DVE).
  2. Channel-min on DVE (per-half for image 0 to cut the kernel head).
  3. Horizontal 15-min-filter: dyadic shift-1/2/4/7 stages on DVE.
  4. PE transpose (identity matmul) into PSUM, 8 blocks per bank, one big
     ScalarE evac per bank -> column layout.
  5. Vertical filter on DVE; for the last two images the final stage is
     split per 128-row slice so the transpose-back/evac/store tail
     pipelines against it.
  6. PE transpose back, ScalarE evacs (the last image's final two run on
     the by-then-idle DVE), f16 stores on SP.

The walrus backend encodes at most ONE sync-wait per instruction and fails
codegen with "Too many sync wait commands" otherwise, while Tile freely
emits several (pool slot reuse, kernel-tail drain).  The post-pass at the
end of _build_nc hoists all but one wait of every instruction onto
single-wait NOPs inserted right before it on the same engine - identical
semantics (the engine sequencer performs the waits in order), and every
instruction then fits the encoding.  CoreSim cannot execute the inserted
NOPs, so the simulator path builds with split_waits=False.
"""

import sys

if "/opt/trn_rl_repo" not in sys.path:
    sys.path.insert(0, "/opt/trn_rl_repo")

import numpy as np

N_CORES = 8
IMGS = 4          # images per core
C = 3
H = W = 512
K = 15
PAD = K // 2      # 7
L = 8             # left pad in filter buffers (>= PAD+1)
PITCH = L + 512 + 8   # 528, padded row/col length
NJ = H // 128     # row tiles
NB = W // 128     # col blocks
PADV = 30000.0    # effective +inf for data in [0,1)

_cache = {}


def _build_nc(split_waits=True):
    import concourse.bass as bass
    import concourse.mybir as mybir
    import concourse.tile as tile
    import concourse.masks as masks

    F16 = mybir.dt.float16
    MIN = mybir.AluOpType.min

    nc = bass.Bass("TRN2", target_bir_lowering=False, debug=False)
    inp = nc.dram_tensor("inp", [IMGS, C, H, W], F16, kind="ExternalInput")
    out = nc.dram_tensor("out", [IMGS, 1, H, W], F16, kind="ExternalOutput")

    with tile.TileContext(nc) as tc:
        with (
            tc.tile_pool(name="const", bufs=1) as cpool,
            tc.tile_pool(name="xpool", bufs=3) as xpool,
            tc.tile_pool(name="ypool", bufs=2) as ypool,
            tc.tile_pool(name="work", bufs=2) as work,
            tc.tile_pool(name="resp", bufs=2) as resp,
            tc.tile_pool(name="vbp", bufs=2) as vbp,
            tc.tile_pool(name="vwork", bufs=2) as vwork,
            tc.tile_pool(name="vresp", bufs=2) as vresp,
            tc.tile_pool(name="opool", bufs=2) as opool,
            tc.tile_pool(name="psum", bufs=3, space="PSUM") as psum,
            tc.tile_pool(name="psum2", bufs=2, space="PSUM") as psum2,
        ):
            ident = cpool.tile([128, 128], F16)
            masks.make_identity(nc, ident[:])
            # pre-warm the ACT activation table (Copy) off the critical path
            warm = cpool.tile([128, 2], F16)
            nc.scalar.copy(warm[:, 0:1], ident[:, 0:1])

            def src_ap(i, c, hh):
                return inp[i, c, 256 * hh : 256 * (hh + 1)].rearrange(
                    "(j p) w -> p j w", p=128
                )

            def loads(i):
                """the 3 channels land concurrently on SP/ACT/Pool queues."""
                xp = xpool.tile([128, NJ, PITCH], F16, tag="xp", name="xp")
                nc.gpsimd.memset(xp[:, :, 0:L], PADV)
                nc.gpsimd.memset(xp[:, :, L + W : PITCH], PADV)
                y = ypool.tile([128, NJ, W], F16, tag="y", name="y")
                z = ypool.tile([128, NJ, W], F16, tag="z", name="z")
                for hh in range(2):
                    jj = slice(2 * hh, 2 * (hh + 1))
                    nc.sync.dma_start(xp[:, jj, L : L + W], src_ap(i, 0, hh))
                    nc.scalar.dma_start(y[:, jj, :], src_ap(i, 1, hh))
                    nc.gpsimd.dma_start(z[:, jj, :], src_ap(i, 2, hh))
                return xp, y, z

            def min_chain(i, src, wpool, rpool, rtag, yz=None,
                          split_final=False):
                """15-wide min filter along last dim of src [128, n, PITCH];
                logical x at [L : L+512].  Returns [128, n, 512] f16."""
                n = src.shape[1]
                if yz is not None:
                    # channel-min; per-half for image 0 so DVE starts as
                    # soon as the first half of the loads lands
                    y, z = yz
                    halves = range(2) if i == 0 else [slice(None)]
                    for hh in halves:
                        jj = (slice(2 * hh, 2 * (hh + 1))
                              if isinstance(hh, int) else hh)
                        nc.vector.tensor_tensor(
                            src[:, jj, L : L + W], src[:, jj, L : L + W],
                            y[:, jj, :], op=MIN,
                        )
                        nc.vector.tensor_tensor(
                            src[:, jj, L : L + W], src[:, jj, L : L + W],
                            z[:, jj, :], op=MIN,
                        )
                s1 = wpool.tile([128, n, PITCH], F16, tag=rtag + "a", name="s1")
                nc.vector.tensor_tensor(
                    s1[:, :, 0:526], src[:, :, 0:526], src[:, :, 1:527], op=MIN
                )
                s2 = wpool.tile([128, n, PITCH], F16, tag=rtag + "b", name="s2")
                nc.vector.tensor_tensor(
                    s2[:, :, 0:524], s1[:, :, 0:524], s1[:, :, 2:526], op=MIN
                )
                s4 = wpool.tile([128, n, PITCH], F16, tag=rtag + "c", name="s4")
                nc.vector.tensor_tensor(
                    s4[:, :, 0:520], s2[:, :, 0:520], s2[:, :, 4:524], op=MIN
                )
                res = rpool.tile([128, n, W], F16, tag=rtag, name="res")
                if split_final:
                    # final combine per 128-slice so downstream per-slice
                    # consumers (t2_out) start before the whole image is done
                    for j in range(NJ):
                        nc.vector.tensor_tensor(
                            res[:, :, 128 * j : 128 * (j + 1)],
                            s4[:, :, 128 * j + 1 : 128 * j + 129],
                            s4[:, :, 128 * j + 8 : 128 * j + 136],
                            op=MIN,
                        )
                else:
                    nc.vector.tensor_tensor(
                        res[:], s4[:, :, 1:513], s4[:, :, 8:520], op=MIN
                    )
                return res

            def make_vb2():
                """column-layout tile holding a PAIR of images, so the
                v-filter processes two images per DVE op."""
                vb2 = vbp.tile([128, 2, NB, PITCH], F16, tag="vb2",
                               name="vb2")
                nc.gpsimd.memset(vb2[:, :, :, 0:L], PADV)
                nc.gpsimd.memset(vb2[:, :, :, L + H : PITCH], PADV)
                return vb2

            def t1(i, res, vb2, hh):
                """row layout -> column layout into half hh of vb2.
                Two 8-block PSUM banks -> two big evacs (less ACT init)."""
                for bp in range(2):
                    pt = psum.tile([128, 2, NJ, 128], F16, tag="pt", name="pt")
                    for bb in range(2):
                        b = 2 * bp + bb
                        for j in range(NJ):
                            nc.tensor.transpose(
                                pt[:, bb, j, :],
                                res[:, j, 128 * b : 128 * (b + 1)],
                                ident[:],
                            )
                    nc.scalar.copy(
                        vb2[:, hh, 2 * bp : 2 * (bp + 1), L : L + H],
                        pt[:].rearrange("p a j w -> p a (j w)"),
                    )

            def t2_out(i, vres):
                """column layout -> row layout, store to HBM."""
                o = opool.tile([128, NJ, W], F16, tag="o", name="o")
                if i < IMGS - 1:
                    # two half-image banks, big evacs, half-image stores
                    for half in range(2):
                        pt = psum.tile([128, 2, NB, 128], F16, tag="pt2",
                                       name="pt2")
                        for jj in range(2):
                            j = 2 * half + jj
                            for b in range(NB):
                                nc.tensor.transpose(
                                    pt[:, jj, b, :],
                                    vres[:, b, 128 * j : 128 * (j + 1)],
                                    ident[:],
                                )
                        nc.scalar.copy(
                            o[:, 2 * half : 2 * (half + 1), :],
                            pt[:].rearrange("p a b w -> p a (b w)"),
                        )
                        nc.sync.dma_start(
                            out[i, 0, 256 * half : 256 * (half + 1)].rearrange(
                                "(j p) w -> p j w", p=128
                            ),
                            o[:, 2 * half : 2 * (half + 1), :],
                        )
                    return
                # last image: per-row-tile chains so the tail pipelines;
                # the last two evacs run on the (by now idle) DVE
                for j in range(NJ):
                    pt = psum2.tile([128, NB, 128], F16, tag="pt3", name="pt3")
                    for b in range(NB):
                        nc.tensor.transpose(
                            pt[:, b, :],
                            vres[:, b, 128 * j : 128 * (j + 1)],
                            ident[:],
                        )
                    flat = pt[:].rearrange("p n w -> p (n w)")
                    if j < 2:
                        nc.scalar.copy(o[:, j, :], flat)
                    else:
                        nc.vector.tensor_copy(o[:, j, :], flat)
                    nc.sync.dma_start(
                        out[i, 0, 128 * j : 128 * (j + 1)].rearrange(
                            "(q p) w -> p q w", p=128
                        ),
                        o[:, j : j + 1, :],
                    )

            def h_stage(i, ld, vb2, hh):
                xp, y, z = ld
                res = min_chain(i, xp, work, resp, "h", yz=(y, z))
                t1(i, res, vb2, hh)

            def v_pair(p, vb2):
                """v-filter over an image pair in one set of DVE ops."""
                s1 = vwork.tile([128, 2, NB, PITCH], F16, tag="va", name="vs1")
                nc.vector.tensor_tensor(
                    s1[:, :, :, 0:526], vb2[:, :, :, 0:526],
                    vb2[:, :, :, 1:527], op=MIN,
                )
                s2 = vwork.tile([128, 2, NB, PITCH], F16, tag="vb", name="vs2")
                nc.vector.tensor_tensor(
                    s2[:, :, :, 0:524], s1[:, :, :, 0:524],
                    s1[:, :, :, 2:526], op=MIN,
                )
                s4 = vwork.tile([128, 2, NB, PITCH], F16, tag="vc", name="vs4")
                nc.vector.tensor_tensor(
                    s4[:, :, :, 0:520], s2[:, :, :, 0:520],
                    s2[:, :, :, 4:524], op=MIN,
                )
                vres2 = vresp.tile([128, 2, NB, W], F16, tag="vres",
                                   name="vres2")
                nc.vector.tensor_tensor(
                    vres2[:], s4[:, :, :, 1:513], s4[:, :, :, 8:520],
                    op=MIN,
                )
                t2_out(2 * p, vres2[:, 0])
                t2_out(2 * p + 1, vres2[:, 1])

            # -- software pipeline, full skew: all H stages then all V --
            ld = [None] * IMGS
            ld[0] = loads(0)
            ld[1] = loads(1)
            vb2a = make_vb2()
            h_stage(0, ld[0], vb2a, 0)
            ld[2] = loads(2)
            h_stage(1, ld[1], vb2a, 1)
            ld[3] = loads(3)
            vb2b = make_vb2()
            h_stage(2, ld[2], vb2b, 0)
            h_stage(3, ld[3], vb2b, 1)
            v_pair(0, vb2a)
            # images 2 and 3 run separate v-chains (split finals) so the
            # transpose-back/store tail pipelines per image
            for i in (2, 3):
                vres = min_chain(i, vb2b[:, i - 2], vwork, vresp, "w",
                                 split_final=True)
                t2_out(i, vres)

    if not split_waits:
        return nc
    # Post-pass: walrus encodes at most ONE sync-wait per instruction.
    # Hoist all but one wait of any multi-wait instruction onto
    # single-wait NOPs inserted just before it on the same engine
    # (identical semantics: the sequencer performs the waits in order).
    import concourse.mybir as mybir

    nsplit = 0
    for bb in nc.main_func.blocks:
        idx = 0
        while idx < len(bb.instructions):
            ins = bb.instructions[idx]
            si = ins.sync_info
            if si is not None and si.on_wait and len(si.on_wait) > 1:
                waits = list(si.on_wait)
                for w in waits[:-1]:
                    nop = mybir.InstNoOp(
                        name=f"W-split-{nsplit}", ins=[], outs=[]
                    )
                    nop.engine = ins.engine
                    nop.sync_info = mybir.SyncInfo(
                        on_wait=[w], on_update=[]
                    )
                    bb.instructions.insert(idx, nop)
                    nsplit += 1
                    idx += 1
                ins.sync_info = mybir.SyncInfo(
                    on_wait=[waits[-1]], on_update=list(si.on_update or [])
                )
            idx += 1
    return nc


def _get_nc():
    if "nc" not in _cache:
        _cache["nc"] = _build_nc()
    return _cache["nc"]


def kernel(I, k):
    from concourse.bass_utils import run_bass_kernel_spmd

    k = int(np.asarray(k))
    assert k == K, f"kernel compiled for k={K}, got {k}"
    I = np.asarray(I)
    B = I.shape[0]
    assert I.shape == (B, C, H, W) and B == N_CORES * IMGS
    I16 = np.ascontiguousarray(I.astype(np.float16))

    nc = _get_nc()
    in_maps = [
        {"inp": I16[c * IMGS : (c + 1) * IMGS]} for c in range(N_CORES)
    ]
    res = run_bass_kernel_spmd(nc, in_maps, list(range(N_CORES))).results
    return np.concatenate(
        [res[c]["out"] for c in range(N_CORES)], axis=0
    ).astype(np.float32)


# revision 27
# speedup vs baseline: 1.0367x; 1.0050x over previous
"""Trainium2 Bass kernel: dark-channel + 15x15 erosion (min-pool, stride 1,
+inf padding), data-parallel over 8 NeuronCores.

Input  I: [32, 3, 512, 512] f32, k: scalar (15)
Output:   [32, 1, 512, 512] f32  (min over channels, then kxk spatial min)

The host converts the input to f16 (values are uniform [0,1); min is pure
selection so f16 keeps rel err ~2e-4, far inside the 2e-2 gate) and the
kernel writes an f16 output that the host upcasts.  This halves HBM traffic
versus f32 in both directions.

Elementwise min exists ONLY on DVE (walrus rejects tensor_tensor min/max on
Pool and min as a DMA compute op), so the kernel is DVE-bound at 10 full-
image f16 passes per image (2 channel-min + 4+4 dyadic erosion stages).
Everything else is arranged to keep DVE 100% fed:

  1. The 3 channels of each image load CONCURRENTLY on the SP/ACT/Pool
     DMA queues (DMA transfer time is charged to the issuing engine, so
     the three streams run in parallel with each other and with DVE).
  2. Channel-min on DVE (per-half for image 0 to cut the kernel head).
  3. Horizontal 15-min-filter: dyadic shift-1/2/4/7 stages on DVE.
  4. PE transpose (identity matmul) into PSUM, 8 blocks per bank, one big
     ScalarE evac per bank -> column layout.
  5. Vertical filter on DVE; for the last two images the final stage is
     split per 128-row slice so the transpose-back/evac/store tail
     pipelines against it.
  6. PE transpose back, ScalarE evacs (the last image's final two run on
     the by-then-idle DVE), f16 stores on SP.

The walrus backend encodes at most ONE sync-wait per instruction and fails
codegen with "Too many sync wait commands" otherwise, while Tile freely
emits several (pool slot reuse, kernel-tail drain).  The post-pass at the
end of _build_nc hoists all but one wait of every instruction onto
single-wait NOPs inserted right before it on the same engine - identical
semantics (the engine sequencer performs the waits in order), and every
instruction then fits the encoding.  CoreSim cannot execute the inserted
NOPs, so the simulator path builds with split_waits=False.
"""

import sys

if "/opt/trn_rl_repo" not in sys.path:
    sys.path.insert(0, "/opt/trn_rl_repo")

import numpy as np

N_CORES = 8
IMGS = 4          # images per core
C = 3
H = W = 512
K = 15
PAD = K // 2      # 7
L = 8             # left pad in filter buffers (>= PAD+1)
PITCH = L + 512 + 8   # 528, padded row/col length
NJ = H // 128     # row tiles
NB = W // 128     # col blocks
PADV = 30000.0    # effective +inf for data in [0,1)

_cache = {}


def _build_nc(split_waits=True):
    import concourse.bass as bass
    import concourse.mybir as mybir
    import concourse.tile as tile
    import concourse.masks as masks

    F16 = mybir.dt.float16
    MIN = mybir.AluOpType.min

    nc = bass.Bass("TRN2", target_bir_lowering=False, debug=False)
    inp = nc.dram_tensor("inp", [IMGS, C, H, W], F16, kind="ExternalInput")
    out = nc.dram_tensor("out", [IMGS, 1, H, W], F16, kind="ExternalOutput")

    with tile.TileContext(nc) as tc:
        with (
            tc.tile_pool(name="const", bufs=1) as cpool,
            tc.tile_pool(name="xpool", bufs=3) as xpool,
            tc.tile_pool(name="ypool", bufs=2) as ypool,
            tc.tile_pool(name="work", bufs=2) as work,
            tc.tile_pool(name="resp", bufs=2) as resp,
            tc.tile_pool(name="vbp", bufs=2) as vbp,
            tc.tile_pool(name="vwork", bufs=2) as vwork,
            tc.tile_pool(name="vresp", bufs=2) as vresp,
            tc.tile_pool(name="opool", bufs=2) as opool,
            tc.tile_pool(name="psum", bufs=3, space="PSUM") as psum,
            tc.tile_pool(name="psum2", bufs=2, space="PSUM") as psum2,
        ):
            ident = cpool.tile([128, 128], F16)
            masks.make_identity(nc, ident[:])
            # pre-warm the ACT activation table (Copy) off the critical path
            warm = cpool.tile([128, 2], F16)
            nc.scalar.copy(warm[:, 0:1], ident[:, 0:1])

            def src_ap(i, c, hh):
                return inp[i, c, 256 * hh : 256 * (hh + 1)].rearrange(
                    "(j p) w -> p j w", p=128
                )

            def loads(i):
                """the 3 channels land concurrently on SP/ACT/Pool queues."""
                xp = xpool.tile([128, NJ, PITCH], F16, tag="xp", name="xp")
                nc.gpsimd.memset(xp[:, :, 0:L], PADV)
                nc.gpsimd.memset(xp[:, :, L + W : PITCH], PADV)
                y = ypool.tile([128, NJ, W], F16, tag="y", name="y")
                z = ypool.tile([128, NJ, W], F16, tag="z", name="z")
                for hh in range(2):
                    jj = slice(2 * hh, 2 * (hh + 1))
                    nc.sync.dma_start(xp[:, jj, L : L + W], src_ap(i, 0, hh))
                    nc.scalar.dma_start(y[:, jj, :], src_ap(i, 1, hh))
                    nc.gpsimd.dma_start(z[:, jj, :], src_ap(i, 2, hh))
                return xp, y, z

            def min_chain(i, src, wpool, rpool, rtag, yz=None,
                          split_final=False):
                """15-wide min filter along last dim of src [128, n, PITCH];
                logical x at [L : L+512].  Returns [128, n, 512] f16."""
                n = src.shape[1]
                if yz is not None:
                    # channel-min; per-half for image 0 so DVE starts as
                    # soon as the first half of the loads lands
                    y, z = yz
                    halves = range(2) if i == 0 else [slice(None)]
                    for hh in halves:
                        jj = (slice(2 * hh, 2 * (hh + 1))
                              if isinstance(hh, int) else hh)
                        nc.vector.tensor_tensor(
                            src[:, jj, L : L + W], src[:, jj, L : L + W],
                            y[:, jj, :], op=MIN,
                        )
                        nc.vector.tensor_tensor(
                            src[:, jj, L : L + W], src[:, jj, L : L + W],
                            z[:, jj, :], op=MIN,
                        )
                s1 = wpool.tile([128, n, PITCH], F16, tag=rtag + "a", name="s1")
                nc.vector.tensor_tensor(
                    s1[:, :, 0:526], src[:, :, 0:526], src[:, :, 1:527], op=MIN
                )
                s2 = wpool.tile([128, n, PITCH], F16, tag=rtag + "b", name="s2")
                nc.vector.tensor_tensor(
                    s2[:, :, 0:524], s1[:, :, 0:524], s1[:, :, 2:526], op=MIN
                )
                s4 = wpool.tile([128, n, PITCH], F16, tag=rtag + "c", name="s4")
                nc.vector.tensor_tensor(
                    s4[:, :, 0:520], s2[:, :, 0:520], s2[:, :, 4:524], op=MIN
                )
                res = rpool.tile([128, n, W], F16, tag=rtag, name="res")
                if split_final:
                    # final combine per 128-slice so downstream per-slice
                    # consumers (t2_out) start before the whole image is done
                    for j in range(NJ):
                        nc.vector.tensor_tensor(
                            res[:, :, 128 * j : 128 * (j + 1)],
                            s4[:, :, 128 * j + 1 : 128 * j + 129],
                            s4[:, :, 128 * j + 8 : 128 * j + 136],
                            op=MIN,
                        )
                else:
                    nc.vector.tensor_tensor(
                        res[:], s4[:, :, 1:513], s4[:, :, 8:520], op=MIN
                    )
                return res

            def make_vb2():
                """column-layout tile holding a PAIR of images, so the
                v-filter processes two images per DVE op."""
                vb2 = vbp.tile([128, 2, NB, PITCH], F16, tag="vb2",
                               name="vb2")
                nc.gpsimd.memset(vb2[:, :, :, 0:L], PADV)
                nc.gpsimd.memset(vb2[:, :, :, L + H : PITCH], PADV)
                return vb2

            def t1(i, res, vb2, hh):
                """row layout -> column layout into half hh of vb2.
                Two 8-block PSUM banks -> two big evacs (less ACT init)."""
                for bp in range(2):
                    pt = psum.tile([128, 2, NJ, 128], F16, tag="pt", name="pt")
                    for bb in range(2):
                        b = 2 * bp + bb
                        for j in range(NJ):
                            nc.tensor.transpose(
                                pt[:, bb, j, :],
                                res[:, j, 128 * b : 128 * (b + 1)],
                                ident[:],
                            )
                    nc.scalar.copy(
                        vb2[:, hh, 2 * bp : 2 * (bp + 1), L : L + H],
                        pt[:].rearrange("p a j w -> p a (j w)"),
                    )

            def t2_out(i, vres):
                """column layout -> row layout, store to HBM."""
                o = opool.tile([128, NJ, W], F16, tag="o", name="o")
                if i < IMGS - 1:
                    # two half-image banks, big evacs, half-image stores
                    for half in range(2):
                        pt = psum.tile([128, 2, NB, 128], F16, tag="pt2",
                                       name="pt2")
                        for jj in range(2):
                            j = 2 * half + jj
                            for b in range(NB):
                                nc.tensor.transpose(
                                    pt[:, jj, b, :],
                                    vres[:, b, 128 * j : 128 * (j + 1)],
                                    ident[:],
                                )
                        nc.scalar.copy(
                            o[:, 2 * half : 2 * (half + 1), :],
                            pt[:].rearrange("p a b w -> p a (b w)"),
                        )
                        nc.sync.dma_start(
                            out[i, 0, 256 * half : 256 * (half + 1)].rearrange(
                                "(j p) w -> p j w", p=128
                            ),
                            o[:, 2 * half : 2 * (half + 1), :],
                        )
                    return
                # last image: per-row-tile chains so the tail pipelines;
                # the last two evacs run on the (by now idle) DVE
                for j in range(NJ):
                    pt = psum2.tile([128, NB, 128], F16, tag="pt3", name="pt3")
                    for b in range(NB):
                        nc.tensor.transpose(
                            pt[:, b, :],
                            vres[:, b, 128 * j : 128 * (j + 1)],
                            ident[:],
                        )
                    flat = pt[:].rearrange("p n w -> p (n w)")
                    if j < 2:
                        nc.scalar.copy(o[:, j, :], flat)
                    else:
                        nc.vector.tensor_copy(o[:, j, :], flat)
                    # spread stores: idle Pool queue for all but the last
                    eng = nc.gpsimd if j < 3 else nc.sync
                    eng.dma_start(
                        out[i, 0, 128 * j : 128 * (j + 1)].rearrange(
                            "(q p) w -> p q w", p=128
                        ),
                        o[:, j : j + 1, :],
                    )

            def h_stage(i, ld, vb2, hh):
                xp, y, z = ld
                res = min_chain(i, xp, work, resp, "h", yz=(y, z))
                t1(i, res, vb2, hh)

            def v_pair(p, vb2):
                """v-filter over an image pair in one set of DVE ops."""
                s1 = vwork.tile([128, 2, NB, PITCH], F16, tag="va", name="vs1")
                nc.vector.tensor_tensor(
                    s1[:, :, :, 0:526], vb2[:, :, :, 0:526],
                    vb2[:, :, :, 1:527], op=MIN,
                )
                s2 = vwork.tile([128, 2, NB, PITCH], F16, tag="vb", name="vs2")
                nc.vector.tensor_tensor(
                    s2[:, :, :, 0:524], s1[:, :, :, 0:524],
                    s1[:, :, :, 2:526], op=MIN,
                )
                s4 = vwork.tile([128, 2, NB, PITCH], F16, tag="vc", name="vs4")
                nc.vector.tensor_tensor(
                    s4[:, :, :, 0:520], s2[:, :, :, 0:520],
                    s2[:, :, :, 4:524], op=MIN,
                )
                vres2 = vresp.tile([128, 2, NB, W], F16, tag="vres",
                                   name="vres2")
                nc.vector.tensor_tensor(
                    vres2[:], s4[:, :, :, 1:513], s4[:, :, :, 8:520],
                    op=MIN,
                )
                t2_out(2 * p, vres2[:, 0])
                t2_out(2 * p + 1, vres2[:, 1])

            # -- software pipeline, full skew: all H stages then all V --
            ld = [None] * IMGS
            ld[0] = loads(0)
            ld[1] = loads(1)
            vb2a = make_vb2()
            h_stage(0, ld[0], vb2a, 0)
            ld[2] = loads(2)
            h_stage(1, ld[1], vb2a, 1)
            ld[3] = loads(3)
            vb2b = make_vb2()
            h_stage(2, ld[2], vb2b, 0)
            h_stage(3, ld[3], vb2b, 1)
            v_pair(0, vb2a)
            # images 2 and 3 run separate v-chains (split finals) so the
            # transpose-back/store tail pipelines per image
            for i in (2, 3):
                vres = min_chain(i, vb2b[:, i - 2], vwork, vresp, "w",
                                 split_final=True)
                t2_out(i, vres)

    if not split_waits:
        return nc
    # Post-pass: walrus encodes at most ONE sync-wait per instruction.
    # Hoist all but one wait of any multi-wait instruction onto
    # single-wait NOPs inserted just before it on the same engine
    # (identical semantics: the sequencer performs the waits in order).
    import concourse.mybir as mybir

    nsplit = 0
    for bb in nc.main_func.blocks:
        idx = 0
        while idx < len(bb.instructions):
            ins = bb.instructions[idx]
            si = ins.sync_info
            if si is not None and si.on_wait and len(si.on_wait) > 1:
                waits = list(si.on_wait)
                for w in waits[:-1]:
                    nop = mybir.InstNoOp(
                        name=f"W-split-{nsplit}", ins=[], outs=[]
                    )
                    nop.engine = ins.engine
                    nop.sync_info = mybir.SyncInfo(
                        on_wait=[w], on_update=[]
                    )
                    bb.instructions.insert(idx, nop)
                    nsplit += 1
                    idx += 1
                ins.sync_info = mybir.SyncInfo(
                    on_wait=[waits[-1]], on_update=list(si.on_update or [])
                )
            idx += 1
    return nc


def _get_nc():
    if "nc" not in _cache:
        _cache["nc"] = _build_nc()
    return _cache["nc"]


def kernel(I, k):
    from concourse.bass_utils import run_bass_kernel_spmd

    k = int(np.asarray(k))
    assert k == K, f"kernel compiled for k={K}, got {k}"
    I = np.asarray(I)
    B = I.shape[0]
    assert I.shape == (B, C, H, W) and B == N_CORES * IMGS
    I16 = np.ascontiguousarray(I.astype(np.float16))

    nc = _get_nc()
    in_maps = [
        {"inp": I16[c * IMGS : (c + 1) * IMGS]} for c in range(N_CORES)
    ]
    res = run_bass_kernel_spmd(nc, in_maps, list(range(N_CORES))).results
    return np.concatenate(
        [res[c]["out"] for c in range(N_CORES)], axis=0
    ).astype(np.float32)
